# revision 15
# baseline (speedup 1.0000x reference)
"""LSTM cell (B=4096, I=H=1024) on 8 Trainium2 NeuronCores via Bass/Tile.

Strategy (2D sharding, 4 batch-splits x 2 gate-column-splits):
  ifgo = x @ w_i + h @ w_h + (b_i + b_h);  i,f,g,o gates -> h_new, c_new.

Per core (p in 0..3 batch quarter, q in 0..1 gate-col half):
  - Operands arrive transposed so the contraction dim (I/H) lands on SBUF
    partitions: xh = [x_p^T ; h_p^T] : [2048, 1024].
  - Weight slice w : [2048, 2048] with columns gate-major ([4 gates x 512
    cols of this q-half]); matmul computes ifgo^T tiles
    [128 gate-cols, 512 batch] in PSUM (lhsT = w tile, rhs = xh tile),
    so the per-gate bias is a per-partition scalar fused into the
    ScalarE activation (sigmoid/tanh) that also evacuates PSUM.
  - Epilogue on VectorE: c_new = f*c + i*g; h_new = o*tanh(c_new).
Matmuls run as float32r (full-rate fp32 PE mode; ~1e-3 abs accuracy on
this problem's scale vs fp64, measured) — set DTYPE_TAG="f32" for exact
fp32 at 1/4 PE rate.

This walrus build rejects >1 sync-wait per instruction, so a post-pass
splits multi-waits into chains of single-wait NoOps (same engine, order
preserved => identical AND semantics).
"""
import numpy as np
from contextlib import ExitStack

import jax

# Persist compiled executables (incl. the walrus-built NEFF wrapped inside)
# across processes so repeat runs skip the multi-minute compile.
jax.config.update("jax_compilation_cache_dir", "/root/.cache/jax_axon")
jax.config.update("jax_persistent_cache_min_entry_size_bytes", 0)
jax.config.update("jax_persistent_cache_min_compile_time_secs", 0)

import concourse.bass as bass
import concourse.mybir as mybir
import concourse.tile as tile
from concourse.bass_utils import run_bass_kernel_spmd

# ---------------------------------------------------------------- config
B, I, H = 4096, 1024, 1024
PG, QG = 4, 2                 # batch splits x gate-col splits = 8 cores
Bp, Hq = B // PG, H // QG     # 1024 batch rows, 512 H cols per core
K = I + H                     # 2048 contraction
KT = K // 128                 # 16 k-tiles
NB = Bp // 512                # 2 batch tiles of 512 (moving free dim)
NC = Hq // 128                # 4 H-col tiles of 128 (psum partition dim)
DTYPE_TAG = "f32r"            # "f32r" (fast) or "f32" (exact, 4x slower PE)

F32 = mybir.dt.float32
AFT = mybir.ActivationFunctionType


# ------------------------------------------------- multi-wait split pass
_wsctr = [0]


def _split_multi_waits(nc):
    """walrus here allows at most ONE sync-wait per instruction; move extra
    waits onto fresh NoOps right before the instruction on the same engine."""
    for fn in nc.m.functions:
        for blk in fn.blocks:
            insts = blk.instructions
            if not any(
                i.sync_info is not None and i.sync_info.on_wait
                and len(i.sync_info.on_wait) > 1
                for i in insts
            ):
                continue
            out = []
            for inst in insts:
                si = inst.sync_info
                if si is not None and si.on_wait and len(si.on_wait) > 1:
                    waits = list(si.on_wait)
                    for w in waits[:-1]:
                        _wsctr[0] += 1
                        nop = mybir.InstNoOp(
                            name=f"wsplit-{_wsctr[0]}", ins=[], outs=[])
                        nop.engine = inst.engine
                        nop.sync_info = mybir.SyncInfo(on_wait=[w], on_update=[])
                        if inst.debug is not None:
                            nop.debug = inst.debug
                        out.append(nop)
                    inst.sync_info = mybir.SyncInfo(
                        on_wait=[waits[-1]], on_update=list(si.on_update or []))
                out.append(inst)
            blk.instructions = out


# ------------------------------------------------------- kernel program
def build_nc(dtype_tag: str = DTYPE_TAG, repeats: int = 1, split: bool = True,
             bench_io: bool = False, mm_order: str = "kg"):
    """One core's program (SPMD across 8 cores). repeats>1 wraps the body in
    a hardware loop — benchmarking only. split=False skips the multi-wait
    split (CoreSim can't see pass-injected NoOps; HW compile needs them).
    bench_io=True swaps the big I/O tensors for internal DRAM (garbage data,
    same HBM traffic) so timing runs skip the ~200 MiB tunnel upload."""
    dt = {"f32": mybir.dt.float32, "f32r": mybir.dt.float32r}[dtype_tag]
    nc = bass.Bass()

    if bench_io:
        xh = nc.dram_tensor("xh", [K, Bp], dt)
        w = nc.dram_tensor("w", [K, 4 * Hq], dt)
        bias = nc.dram_tensor("bias", [4 * Hq], F32)
        c_in = nc.dram_tensor("c", [Hq, Bp], F32)
        h_out = nc.dram_tensor("h_out", [Hq, Bp], F32)
        c_out = nc.dram_tensor("c_out", [Hq, Bp], F32)
        nc.declare_dram_parameter("bench_in", [1, 1], F32, isOutput=False)
        done = nc.declare_dram_parameter("done", [1, 1], F32, isOutput=True)
    else:
        xh = nc.declare_dram_parameter("xh", [K, Bp], dt, isOutput=False)
        w = nc.declare_dram_parameter("w", [K, 4 * Hq], dt, isOutput=False)
        bias = nc.declare_dram_parameter("bias", [4 * Hq], F32, isOutput=False)
        c_in = nc.declare_dram_parameter("c", [Hq, Bp], F32, isOutput=False)
        h_out = nc.declare_dram_parameter("h_out", [Hq, Bp], F32, isOutput=True)
        c_out = nc.declare_dram_parameter("c_out", [Hq, Bp], F32, isOutput=True)
        done = None

    xh_r = xh.rearrange("(kt p) b -> p kt b", p=128)            # [128,16,1024]
    w_r = w.rearrange("(kt p) (g cc n) -> p kt g cc n",
                      p=128, g=4, cc=NC)                        # [128,16,4,4,128]

    with tile.TileContext(nc) as tc, ExitStack() as ctx:
        xh_pool = ctx.enter_context(tc.tile_pool(name="xh", bufs=1))
        w_pool = ctx.enter_context(tc.tile_pool(name="w", bufs=2))
        b_pool = ctx.enter_context(tc.tile_pool(name="bias", bufs=1))
        c_pool = ctx.enter_context(tc.tile_pool(name="cin", bufs=3))
        psum = ctx.enter_context(tc.tile_pool(name="ps", bufs=2, space="PSUM"))
        epi = ctx.enter_context(tc.tile_pool(name="epi", bufs=3))

        # resident: all of xh^T (8 MiB, one tile per k so consumers only wait
        # on their own chunk and the 16 DMAs spread across HW queues) and the
        # 16 per-partition biases. In bench mode the xh load moves inside the
        # repeat loop so per-iteration time covers the full workload.
        xh_t = [None] * KT

        def load_xh():
            for k in range(KT):
                t = xh_pool.tile([128, Bp], dt, tag=f"xh{k}", name=f"xh{k}")
                nc.sync.dma_start(out=t[:], in_=xh_r[:, k, :])
                xh_t[k] = t

        if not bench_io:
            load_xh()
        bias_t = [[None] * NC for _ in range(4)]
        for g in range(4):
            for c in range(NC):
                bt_ = b_pool.tile([128, 1], F32, tag=f"b{g}{c}")
                nc.sync.dma_start(
                    out=bt_[:],
                    in_=bias[g * Hq + c * 128:g * Hq + (c + 1) * 128]
                    .rearrange("(p x) -> p x", x=1))
                bias_t[g][c] = bt_

        def body(_iv=None):
            if bench_io:
                load_xh()
            for c in range(NC):
                w_ts = []
                for g in range(4):
                    wt = w_pool.tile([128, KT, 128], dt, tag=f"w{g}")
                    nc.sync.dma_start(out=wt[:], in_=w_r[:, :, g, c, :])
                    w_ts.append(wt)
                for bt in range(NB):
                    # k-outer / gate-inner: consecutive MMs share the moving
                    # operand and alternate PSUM banks — measurably better
                    # LDW/MM pipelining than 16-deep same-bank runs.
                    ps = [psum.tile([128, 512], F32, tag=f"ps{g}",
                                    name=f"ps{g}_{c}_{bt}") for g in range(4)]
                    if mm_order == "kg":
                        for k in range(KT):
                            for g in range(4):
                                nc.tensor.matmul(
                                    ps[g][:], w_ts[g][:, k, :],
                                    xh_t[k][:, bt * 512:(bt + 1) * 512],
                                    start=(k == 0), stop=(k == KT - 1))
                    else:
                        for g in range(4):
                            for k in range(KT):
                                nc.tensor.matmul(
                                    ps[g][:], w_ts[g][:, k, :],
                                    xh_t[k][:, bt * 512:(bt + 1) * 512],
                                    start=(k == 0), stop=(k == KT - 1))
                    ct = c_pool.tile([128, 512], F32, tag="ct")
                    nc.sync.dma_start(
                        out=ct[:],
                        in_=c_in[c * 128:(c + 1) * 128, bt * 512:(bt + 1) * 512])
                    i_s = epi.tile([128, 512], F32, tag="i_s")
                    nc.scalar.activation(i_s[:], ps[0][:], AFT.Sigmoid,
                                         bias=bias_t[0][c][:])
                    f_s = epi.tile([128, 512], F32, tag="f_s")
                    nc.scalar.activation(f_s[:], ps[1][:], AFT.Sigmoid,
                                         bias=bias_t[1][c][:])
                    g_t = epi.tile([128, 512], F32, tag="g_t")
                    nc.scalar.activation(g_t[:], ps[2][:], AFT.Tanh,
                                         bias=bias_t[2][c][:])
                    o_s = epi.tile([128, 512], F32, tag="o_s")
                    nc.scalar.activation(o_s[:], ps[3][:], AFT.Sigmoid,
                                         bias=bias_t[3][c][:])
                    t1 = epi.tile([128, 512], F32, tag="t1")
                    nc.vector.tensor_mul(t1[:], f_s[:], ct[:])
                    t2 = epi.tile([128, 512], F32, tag="t2")
                    nc.vector.tensor_mul(t2[:], i_s[:], g_t[:])
                    cn = epi.tile([128, 512], F32, tag="cn")
                    nc.vector.tensor_add(cn[:], t1[:], t2[:])
                    th = epi.tile([128, 512], F32, tag="th")
                    nc.scalar.activation(th[:], cn[:], AFT.Tanh)
                    hn = epi.tile([128, 512], F32, tag="hn")
                    nc.vector.tensor_mul(hn[:], o_s[:], th[:])
                    nc.sync.dma_start(
                        out=c_out[c * 128:(c + 1) * 128, bt * 512:(bt + 1) * 512],
                        in_=cn[:])
                    nc.sync.dma_start(
                        out=h_out[c * 128:(c + 1) * 128, bt * 512:(bt + 1) * 512],
                        in_=hn[:])

        if repeats == 1:
            body()
        else:
            with tc.For_i(0, repeats, 1) as iv:
                body(iv)
        if done is not None:
            nc.sync.dma_start(out=done[:], in_=bias_t[0][0][0:1, 0:1])

    if split:
        _split_multi_waits(nc)
    return nc


# ----------------------------------------------------------- host glue
def shard_inputs(input, h_t, c_t, w_i, w_h, b_i, b_h):
    xT = np.ascontiguousarray(input.T, dtype=np.float32)   # [I, B]
    hT = np.ascontiguousarray(h_t.T, dtype=np.float32)     # [H, B]
    wcat = np.concatenate([w_i, w_h], axis=0).astype(np.float32, copy=False)
    bsum = (b_i + b_h).astype(np.float32)
    w_q, bias_q = [], []
    for q in range(QG):
        w_q.append(np.ascontiguousarray(
            wcat.reshape(K, 4, QG, Hq)[:, :, q, :].reshape(K, 4 * Hq)))
        bias_q.append(np.ascontiguousarray(
            bsum.reshape(4, QG, Hq)[:, q, :].reshape(4 * Hq)))
    in_maps = []
    for p in range(PG):
        xh_p = np.concatenate(
            [xT[:, p * Bp:(p + 1) * Bp], hT[:, p * Bp:(p + 1) * Bp]], axis=0)
        for q in range(QG):
            cT = np.ascontiguousarray(
                c_t[p * Bp:(p + 1) * Bp, q * Hq:(q + 1) * Hq].T)
            in_maps.append({"xh": xh_p, "w": w_q[q], "bias": bias_q[q], "c": cT})
    return in_maps


def unshard_outputs(results):
    h_new = np.empty((B, H), np.float32)
    c_new = np.empty((B, H), np.float32)
    for p in range(PG):
        for q in range(QG):
            r = results[p * QG + q]
            h_new[p * Bp:(p + 1) * Bp, q * Hq:(q + 1) * Hq] = r["h_out"].T
            c_new[p * Bp:(p + 1) * Bp, q * Hq:(q + 1) * Hq] = r["c_out"].T
    return h_new, c_new


_nc_cache = {}


def _get_nc(dtype_tag=DTYPE_TAG, repeats=1):
    key = (dtype_tag, repeats)
    if key not in _nc_cache:
        _nc_cache[key] = build_nc(dtype_tag, repeats)
    return _nc_cache[key]


def kernel(input, h_t, c_t, w_i, w_h, b_i, b_h):
    args = [np.asarray(a, dtype=np.float32)
            for a in (input, h_t, c_t, w_i, w_h, b_i, b_h)]
    in_maps = shard_inputs(*args)
    nc = _get_nc()
    res = run_bass_kernel_spmd(nc, in_maps, list(range(PG * QG)))
    return unshard_outputs(res.results)
